# revision 50
# baseline (speedup 1.0000x reference)
"""Trainium2 Bass kernel for nn_EquivEncoder (RBF SetConv grid encoder).

Math:
    grid is a 64x64 tensor-product grid, so the RBF Gram factorizes:
        Gram[(k,j), n] = Ky[k,n] * Kx[j,n]
        Kx[j,n] = exp(s*(xs_j - X[n,0])^2),  Ky[k,n] = exp(s*(ys_k - X[n,1])^2)
        s = -0.5 / l^2
    Then for channels EY = [1, Y0, Y1]:
        FM[b,k,j,c] = sum_n Ky[k,n] * Kx[j,n] * EY[n,c]
    Output: [dens, FM1/dens, FM2/dens] reshaped to (B, 3, 64, 64).

Device algorithm (per core, 2 batches of 1024 context points = 8 n-tiles
of 128, grouped as 4 "quads" of 4 tiles):
    - One DMA delivers BIGT (17, 1024): cols 0:512 hold per-quad point
      data (rows [sq-slots x,y]*4 / [x,y]*4 / ones), cols 512:1024 hold a
      constant R block encoding s*(coord - grid)^2 per tile slot. One
      in-place DVE multiply per batch squares rows 0:8.
    - per quad: ONE K=17 N=512 matmul -> s*d^2 for both coords of all 4
      tiles (PSUM, full bank); ONE exp scattered into 4 block slots of T;
      ONE broadcast multiply appends Kx*Y0|Kx*Y1 per tile.
    - per tile: one K=128 N=192 matmul accumulates FM (64,192) in PSUM.
    - epilogue: reciprocal of dens, broadcast divide, two DMAs out.
    - a PE "delay line" of 1x1 dummy matmuls keeps the in-order PE
      sequencer from stalling on the input DMA (a stall resets the PE
      p-state ramp and halves matmul throughput).

Sharding: pure data parallel, 2 of 16 batches per core across 8 cores.
"""

import sys

if "/opt/trn_rl_repo" not in sys.path:
    sys.path.insert(0, "/opt/trn_rl_repo")

import numpy as np

N_X = 64
N_Y = 64
N_CTX = 1024
N_CORES = 8
B_TOTAL = 16
B_PER_CORE = B_TOTAL // N_CORES
X_RANGE = (-10.0, 10.0)
Y_RANGE = (-10.0, 10.0)

NQUAD = B_PER_CORE * 2  # 4 quads of 4 tiles of 128 points
WTOT = 1024  # 512 data cols + 512 R cols

_NC_CACHE = {}
_RH_CACHE = {}


def _build_nc(nops=8, nfp32=2):
    import concourse.bacc as bacc
    import concourse.tile as tile
    import concourse.mybir as mybir

    f32 = mybir.dt.float32
    EXP = mybir.ActivationFunctionType.Exp
    COPY = mybir.ActivationFunctionType.Copy

    nc = bacc.Bacc(
        "TRN2",
        target_bir_lowering=False,
        debug=False,
        num_devices=N_CORES,
    )
    f16 = mybir.dt.float16
    XA_d = nc.declare_dram_parameter("XA", [17, 1024], f32, isOutput=False)
    RH_d = nc.declare_dram_parameter("RH", [128, 512], f16, isOutput=False)
    YA_d = nc.declare_dram_parameter(
        "YA", [128, B_PER_CORE, 8, 2], f32, isOutput=False
    )
    out_d = nc.declare_dram_parameter(
        "out", [B_PER_CORE, 3, N_Y, N_X], f32, isOutput=True
    )

    from concourse.tile import add_dep_helper

    with tile.TileContext(nc) as tc:
        with (
            tc.tile_pool(name="const", bufs=1) as constp,
            tc.tile_pool(name="work", bufs=4) as workp,
            tc.tile_pool(name="psum", bufs=3, space="PSUM") as psump,
            tc.tile_pool(name="fmp", bufs=2, space="PSUM") as fmp,
            tc.tile_pool(name="outp", bufs=2) as outp,
        ):
            # PE delay-line: tiny 1x1 matmuls on a constant tile hold the
            # in-order PE sequencer back until the input DMA + squares
            # land, so no real matmul ever stalls on a wait (a stall
            # resets the PE p-state ramp and halves matmul throughput).
            if nops:
                dummy = constp.tile([1, 2], f32)
                nc.gpsimd.memset(dummy[:], 1.0)
                dps = fmp.tile([1, 1], f32, tag="fmd")
                for _ in range(nops):
                    nc.tensor.matmul(
                        dps[:],
                        dummy[0:1, 0:1],
                        dummy[0:1, 1:2],
                        start=True,
                        stop=True,
                    )

            # fp16 two-plane split of the stationary data: BH rows
            # [A1@0 | A2@32 | A1@64 | A2@96] (zero padding between) paired
            # with RH rows [R1 | R1 | R2 | R2] computes (A1+A2)*(R1+R2) =
            # full fp32-accuracy products at fp16 matmul speed (1 cyc/row
            # vs 4 for fp32).
            BH = constp.tile([128, 512], f16)
            nc.vector.memset(BH[:], 0.0)
            BIGT = constp.tile([17, 1024], f32)
            nc.sync.dma_start(BIGT[:], XA_d[:])
            RHs = constp.tile([128, 512], f16)
            nc.gpsimd.dma_start(RHs[:], RH_d[:])

            # Y channel scalars, partition-aligned with n-tiles: (128, b, t, d)
            Ysc = constp.tile([128, B_PER_CORE, 8, 2], f32)
            nc.sync.dma_start(Ysc[:], YA_d[:])

            # Phase 1: per quad, square + fp16-split prep + one w-matmul
            # + one exp + one product op
            Ts = {}
            for qidx in range(NQUAD):
                b, Q = divmod(qidx, 2)
                ch = slice(qidx * 128, (qidx + 1) * 128)
                seg = BIGT[0:8, ch]
                nc.vector.tensor_mul(seg, seg, seg)
                P4 = psump.tile([128, 512], f32, tag="P4")
                if qidx < nfp32:
                    # first two quads: direct fp32 matmul against the fp32
                    # R block (cols 512:1024 of the same DMA) -- slower on
                    # PE but PE is otherwise idle, and it starts the serial
                    # ACT exp chain ~0.6us earlier
                    nc.tensor.matmul(
                        P4[:], BIGT[:, ch], BIGT[:, 512:1024],
                        start=True, stop=True,
                    )
                else:
                    nc.vector.tensor_copy(BH[0:17, ch], BIGT[:, ch])
                    nc.vector.tensor_sub(
                        BH[32:49, ch], BIGT[:, ch], BH[0:17, ch]
                    )
                    nc.gpsimd.tensor_copy(BH[64:128, ch], BH[0:64, ch])
                    nc.tensor.matmul(
                        P4[:], BH[:, ch], RHs[:], start=True, stop=True
                    )

                # T = [Ky_g | Kx_g | prods_g] x 4 blocks of 256
                T = workp.tile([128, 1024], f32, tag="T")
                Ts[qidx] = T
                Tv = T.rearrange("p (g h) -> p g h", h=256)
                nc.scalar.activation(Tv[:, :, 0:128], P4[:], EXP)

                # all 4 tiles' Y products in one op:
                # out[p, g, j, c] = Kx_g[p, j] * Y_g[p, c]
                prod_out = Tv[:, :, 128:256].rearrange("p g (j c) -> p g j c", c=2)
                kx_b = Tv[:, :, 64:128].broadcast_to((128, 4, 64, 2))
                y_b = (
                    Ysc[:, b, 4 * Q : 4 * Q + 4, :]
                    .broadcast_to((128, 4, 2, 64))
                    .transpose([0, 1, 3, 2])
                )
                nc.vector.tensor_mul(prod_out, kx_b, y_b)

            # Phase 2: accumulate FM. The dens (N=64) and prods (N=128)
            # columns accumulate in separate PSUM banks, dens chains first,
            # so the reciprocal/dens output complete mid-kernel and only
            # one divide + DMA is exposed after the last matmul.
            fmd = {}
            fmps = {}
            for b in range(B_PER_CORE):
                fmd_t = fmp.tile([64, 64], f32, tag="fmd", name=f"fmd{b}")
                fmd[b] = fmd_t
                fmp_t = fmp.tile([64, 128], f32, tag="fmp", name=f"fmp{b}")
                fmps[b] = fmp_t

            def fd(b, ts):
                for t in ts:
                    Q, g = divmod(t, 4)
                    base = 256 * g
                    nc.tensor.matmul(
                        fmd[b][:],
                        Ts[2 * b + Q][:, base : base + 64],
                        Ts[2 * b + Q][:, base + 64 : base + 128],
                        start=(t == 0),
                        stop=(t == 7),
                    )

            def fp(b, ts):
                for t in ts:
                    Q, g = divmod(t, 4)
                    base = 256 * g
                    nc.tensor.matmul(
                        fmps[b][:],
                        Ts[2 * b + Q][:, base : base + 64],
                        Ts[2 * b + Q][:, base + 128 : base + 256],
                        start=(t == 0),
                        stop=(t == 7),
                    )

            fd(0, range(8))
            fd(1, range(4))
            fp(0, range(4))
            fd(1, range(4, 8))
            fp(0, range(4, 8))
            fp(1, range(8))
            # dens epilogues right away (recip on DVE, copy on ACT)
            recips = {}
            for b in range(B_PER_CORE):
                recip = workp.tile([64, 64], f32, tag="recip")
                recips[b] = recip
                nc.vector.reciprocal(recip[:], fmd[b][:])
                oden = outp.tile([64, 64], f32, tag="oden")
                nc.scalar.activation(oden[:], fmd[b][:], COPY)
                nc.sync.dma_start(out_d[b, 0], oden[:])
            # epilogues: divide channels 1,2 by dens
            for b in range(B_PER_CORE):
                osb = outp.tile([64, 128], f32, tag="osb")
                nc.vector.tensor_mul(
                    osb.rearrange("k (c j) -> k c j", c=2),
                    fmps[b].rearrange("k (j c) -> k c j", c=2),
                    recips[b].broadcast_to((64, 64, 2)).transpose([0, 2, 1]),
                )
                nc.sync.dma_start(
                    out_d[b, 1:3].rearrange("c k j -> k c j"),
                    osb.rearrange("p (c j) -> p c j", c=2),
                )
    nc.compile()
    return nc


def _host_inputs(X, Y, log_l_scale):
    """Per-core input arrays: BIGT source (17, 1024) and Y scalars."""
    s = -0.5 * float(np.exp(-2.0 * np.float64(log_l_scale)))
    xs = np.linspace(X_RANGE[0], X_RANGE[1], N_X, dtype=np.float32).astype(np.float64)
    ys = np.linspace(Y_RANGE[1], Y_RANGE[0], N_Y, dtype=np.float32).astype(np.float64)
    # R block (17, 512): tile-slot g occupies cols 128g:128g+128 with
    # [y-grid | x-grid] halves; lhsT rows are
    # [x^2,y^2 per slot (0:8) | x,y per slot (8:16) | ones (16)].
    # Shipped as two stacked fp16 planes R1=f16(R), R2=f16(R-R1).
    R = np.zeros((17, 512), np.float64)
    for g in range(4):
        c = 128 * g
        R[2 * g + 1, c : c + 64] = s
        R[8 + 2 * g + 1, c : c + 64] = -2.0 * s * ys
        R[16, c : c + 64] = s * ys**2
        R[2 * g, c + 64 : c + 128] = s
        R[8 + 2 * g, c + 64 : c + 128] = -2.0 * s * xs
        R[16, c + 64 : c + 128] = s * xs**2
    R1 = R.astype(np.float16)
    R2 = (R - R1.astype(np.float64)).astype(np.float16)
    RH = np.zeros((128, 512), np.float16)
    RH[0:17] = R1
    RH[32:49] = R1
    RH[64:81] = R2
    RH[96:113] = R2

    _RH_CACHE["rh"] = RH
    xa_list, ya_list = [], []
    for i in range(N_CORES):
        Xc = X[i * B_PER_CORE : (i + 1) * B_PER_CORE]  # (2, 1024, 2)
        Yc = Y[i * B_PER_CORE : (i + 1) * B_PER_CORE]
        XA = np.empty((17, 1024), np.float32)
        # quad qidx = 2b+Q at cols 128qidx:128qidx+128; tile g of the quad
        # is global tile t = 4Q+g of batch b
        Xq = Xc.reshape(B_PER_CORE, 2, 4, 128, 2)  # (b, Q, g, m, d)
        for qidx in range(NQUAD):
            b, Q = divmod(qidx, 2)
            c = 128 * qidx
            for g in range(4):
                XA[2 * g, c : c + 128] = Xq[b, Q, g, :, 0]
                XA[2 * g + 1, c : c + 128] = Xq[b, Q, g, :, 1]
                XA[8 + 2 * g, c : c + 128] = Xq[b, Q, g, :, 0]
                XA[8 + 2 * g + 1, c : c + 128] = Xq[b, Q, g, :, 1]
        XA[16, 0:512] = 1.0
        XA[:, 512:1024] = R.astype(np.float32)
        # (b, t*128+p, d) -> (p, b, t, d)
        YA = np.ascontiguousarray(
            Yc.reshape(B_PER_CORE, 8, 128, 2).transpose(2, 0, 1, 3),
            dtype=np.float32,
        )
        xa_list.append(XA)
        ya_list.append(YA)
    return xa_list, ya_list


def _run(X, Y, log_l_scale, trace=False, **kw):
    from concourse.bass_utils import run_bass_kernel_spmd

    X = np.ascontiguousarray(X, dtype=np.float32)
    Y = np.ascontiguousarray(Y, dtype=np.float32)
    xa_list, ya_list = _host_inputs(X, Y, log_l_scale)
    in_maps = [
        {"XA": xa_list[i], "YA": ya_list[i], "RH": _RH_CACHE["rh"]}
        for i in range(N_CORES)
    ]
    if "nc" not in _NC_CACHE:
        _NC_CACHE["nc"] = _build_nc()
    res = run_bass_kernel_spmd(
        _NC_CACHE["nc"], in_maps, list(range(N_CORES)), trace=trace, **kw
    )
    out = np.concatenate([res.results[i]["out"] for i in range(N_CORES)], axis=0)
    return out, res


def kernel(X, Y, log_l_scale):
    out, _ = _run(X, Y, log_l_scale)
    return out.astype(np.float32)
